# revision 42
# baseline (speedup 1.0000x reference)
"""Tropical (max-plus) linear kernel for Trainium2 via log-sum-exp matmul.

out[b, o] = max_i (W[o, i] + x[b, i]),  x: [512, 1024] f32, W: [512, 1024] f32.

Identity: max_i(W+x) = (1/t)*log(sum_i e^{t(x-c)} * e^{tW}) + c - smoothing,
so the max-plus contraction becomes a real bf16 GEMM on the Tensor engine
instead of a broadcast-add + reduce-max on the Vector engine (the 792us
baseline). Smoothing error <= ln(#near-ties)/t; with t=25, c=4 the exact
end-to-end numerics (fp16 inputs, bf16 exp, f32 psum) give max abs err 0.063
vs the 0.108 tolerance (2e-2 * absmax), verified against the reference on
all 512x512 outputs. The c-shift keeps e^{t(x-c)} inside bf16 range for
every input that can influence a row max; it is folded into the host-side
fp16 packing (x - 4.0) and added back by the final affine.

Sharding (8 NeuronCores, SPMD): grid of NO_SH out-shards x NB_SH batch-shards
(NO_SH*NB_SH = 8; default 4x2). Host packs each core's x/W slices k-major-
transposed into one fp16 tensor and reassembles the per-core [O_SH, B_SH]
outputs.

Per-core body (~12 instructions, default config):
  DMA in  : packed [128, KC, B_SH+O_SH] f16, split over 2 DGE queues
  Vector  : ee = Schraudolph exp: ONE int16 tensor_scalar writes the bf16
            BIT PATTERNS of e^{T*in} directly (bits = z*128 + 16250.5,
            ~3% rel err -> +-0.0024 on the output after /T)
  Tensor  : psum[O_SH, B_SH] = sum_c ee[:,c,W-part].T @ ee[:,c,x-part]
  Vector  : Schraudolph log fused with /T + C: ONE tensor_scalar on the
            int32-bitcast psum;  DMA out [O_SH, B_SH] f32 on gpsimd queue

Engine-queue discipline matters most: the out-DMA must NOT share the SP DMA
queue with the in-DMA, or head-of-line blocking serializes iterations
(measured 15.4 us vs 4 us). The Scalar-engine Exp/Ln path (KEXP=act,
KLOG=ln) is kept as a fallback config; the bit-trick path frees the Scalar
engine entirely and fuses the whole epilogue into one Vector op.

Timing note: on this axon-proxied setup, per-call wall time scales with NEFF
size (payload upload), so python-unrolled nrep-differencing measures upload
cost (~1.2 ms/"iter"), not device time. build_nc(nrep>1) therefore wraps the
body in a tc.For_i hardware loop (constant NEFF size; trip count differencing
isolates true device exec time). The body is unrolled BODY_UNROLL x inside
the loop so the per-trip all-engine barrier (~10 us) amortizes away.
"""

import os

import numpy as np  # noqa: E402

import concourse.bacc as bacc
import concourse.tile as tile
from concourse import mybir
from concourse.bass_utils import run_bass_kernel_spmd

B, IN, OUT = 512, 1024, 512
NCORES = 8
KC = IN // 128  # 8 k-chunks of 128 partitions

T = 25.0
# c=3.0 with the -2.0 clamp keeps every exp value, bf16 product, and f32
# psum term in NORMAL float range (min product ~2e-27 vs denormal threshold
# 1.2e-38) -- denormal operands measurably slow the PE on this hardware.
C = float(os.environ.get("KC0", "3.0"))
XCLAMP = float(os.environ.get("KCLAMP", "-2.0"))

F32 = mybir.dt.float32
F16 = mybir.dt.float16
BF16 = mybir.dt.bfloat16
EXP = mybir.ActivationFunctionType.Exp
LN = mybir.ActivationFunctionType.Ln
COPY = mybir.ActivationFunctionType.Copy
MULT = mybir.AluOpType.mult
ADD = mybir.AluOpType.add

BODY_UNROLL = int(os.environ.get("KUNROLL", "16"))
KBUFS = int(os.environ.get("KBUFS", "4"))
KPSBUFS = int(os.environ.get("KPSBUFS", "8"))
KAFF = os.environ.get("KAFF", "act")  # dve | act (only used with KLOG=ln)
KSPLIT = int(os.environ.get("KSPLIT", "2"))  # in-DMA split count
KODMAENG = os.environ.get("KODMAENG", "gpsimd")  # sync | gpsimd
KPIPE = int(os.environ.get("KPIPE", "0"))  # software-pipeline unrolled bodies
KLOG = os.environ.get("KLOG", "bits")  # ln | bits (Schraudolph log on DVE)
KEXP = os.environ.get("KEXP", "split")  # act | dve | dvew | split
KSHARD = os.environ.get("KSHARD", "o4b2")  # o8 | o4b2
KABLATE = os.environ.get("KABLATE", "")  # nodma|noexp|nomm|noout|empty (perf probes)
KSTAG = int(os.environ.get("KSTAG", "0"))  # For_i staggered_reset
KDQ = os.environ.get("KDQ", "ss")  # in-DMA queue rotation: ss=sync/scalar, sg=sync/gpsimd

NO_SH, NB_SH = (8, 1) if KSHARD == "o8" else (4, 2)
O_SH = OUT // NO_SH
B_SH = B // NB_SH
M = B_SH + O_SH  # packed columns per k-chunk

# legacy name used by test.py's sim path
O_PER_CORE = O_SH


def build_nc(nrep: int = 1) -> bacc.Bacc:
    nc = bacc.Bacc("TRN2", num_devices=NCORES)
    # inh[p, c*M + b]      = f16(x[b0 + b, c*128 + p] - C)   b in [0, B_SH)
    # inh[p, c*M + B_SH+o] = f16(W[o0 + o, c*128 + p])       o in [0, O_SH)
    inh = nc.dram_tensor("inh", [128, KC * M], F16, kind="ExternalInput")
    out = nc.dram_tensor("out", [O_SH, B_SH], F32, kind="ExternalOutput")
    # Proof the timing loop really ran: per-iteration counter, read back by
    # the harness and checked against nrep (the body itself is idempotent,
    # so output correctness alone can't detect a broken/short loop).
    iters = nc.dram_tensor("iters", [1, 1], F32, kind="ExternalOutput")

    with tile.TileContext(nc) as tc:
        with (
            tc.tile_pool(name="cnt", bufs=1) as cnt,
            tc.tile_pool(name="ip", bufs=KBUFS) as ip,
            tc.tile_pool(name="ep", bufs=KBUFS) as ep,
            tc.tile_pool(name="op", bufs=KBUFS) as op,
            tc.tile_pool(name="ps", bufs=KPSBUFS, space="PSUM") as ps,
        ):
            counter = cnt.tile([1, 1], F32, tag="cnt", name="cnt")
            nc.gpsimd.memset(counter[:, :], 0.0)

            def front():
                if KABLATE == "empty":
                    return None
                ts = ip.tile([128, KC * M], F16, tag="ts", name="ts")
                if KABLATE == "tinydma":
                    nc.sync.dma_start(out=ts[:, 0:128], in_=inh[:, 0:128])
                elif KABLATE == "nodma":
                    pass
                elif KSPLIT == 1:
                    nc.sync.dma_start(out=ts, in_=inh[:, :])
                else:
                    step = KC * M // KSPLIT
                    rot = (
                        [nc.sync, nc.gpsimd, nc.scalar]
                        if KDQ == "sg"
                        else [nc.sync, nc.scalar, nc.gpsimd]
                    )
                    for s in range(KSPLIT):
                        eng = rot[s % 3]
                        eng.dma_start(
                            out=ts[:, s * step : (s + 1) * step],
                            in_=inh[:, s * step : (s + 1) * step],
                        )
                ee = ep.tile([128, KC * M], BF16, tag="ee", name="ee")
                if KABLATE == "noexp":
                    ee = ts.bitcast(BF16) if hasattr(ts, "bitcast") else ts
                    ee3 = ts[:, :].rearrange("p (c m) -> p c m", c=KC)
                    psum = ps.tile([O_SH, B_SH], F32, tag="ps", name="ps")
                    for c in range(KC):
                        nc.tensor.matmul(
                            psum[:, :], ee3[:, c, B_SH : B_SH + O_SH],
                            ee3[:, c, 0:B_SH], start=(c == 0), stop=(c == KC - 1),
                        )
                    return psum
                if KEXP == "act":
                    nc.scalar.activation(ee[:, :], ts[:, :], EXP, bias=0.0, scale=T)
                elif KEXP in ("split", "split3"):
                    # Split the exp wall time across engines: Scalar does the
                    # leading k-chunks with exact table exp while Vector (and
                    # for split3, GpSimd) do the rest with the Schraudolph
                    # bit trick, all in parallel.
                    h = (KC // 2) * M if KEXP == "split" else 3 * M
                    nc.scalar.activation(
                        ee[:, 0:h], ts[:, 0:h], EXP, bias=0.0, scale=T
                    )
                    es1 = float(T * np.log2(np.e) * 128.0)
                    es2 = float((127.0 - 0.0430357) * 128.0)
                    eei = ee[:, :].bitcast(mybir.dt.int16)
                    h2 = KC * M if KEXP == "split" else 6 * M
                    nc.vector.tensor_scalar(
                        eei[:, h:h2], ts[:, h:h2], es1, es2, MULT, ADD
                    )
                    if KEXP == "split3":
                        nc.gpsimd.tensor_scalar(
                            eei[:, h2 : KC * M], ts[:, h2 : KC * M],
                            es1, es2, MULT, ADD,
                        )
                else:
                    # Schraudolph exp, built directly as bf16 bit patterns:
                    # e^{T*y} = 2^{T*y*log2(e)}; bf16 bits ~ z*128 + (127-m)*128
                    # (~3% rel err -> +-0.0024 after the final /T; host packing
                    # clamps x-C at -3.4 so the int16 range can't wrap).
                    es1 = float(T * np.log2(np.e) * 128.0)
                    es2 = float((127.0 - 0.0430357) * 128.0)
                    eei = ee[:, :].bitcast(mybir.dt.int16)
                    if KEXP == "dve":
                        nc.vector.tensor_scalar(
                            eei, ts[:, :], es1, es2, MULT, ADD
                        )
                    else:  # dvew: x-part exact on ACT, W-part Schraudolph on DVE
                        t3 = ts[:, :].rearrange("p (c m) -> p c m", c=KC)
                        x3 = ee[:, :].rearrange("p (c m) -> p c m", c=KC)
                        nc.scalar.activation(
                            x3[:, :, 0:B_SH], t3[:, :, 0:B_SH], EXP,
                            bias=0.0, scale=T,
                        )
                        w3 = eei.rearrange("p (c m) -> p c m", c=KC)
                        nc.vector.tensor_scalar(
                            w3[:, :, B_SH : B_SH + O_SH],
                            t3[:, :, B_SH : B_SH + O_SH],
                            es1, es2, MULT, ADD,
                        )
                e3 = ee[:, :].rearrange("p (c m) -> p c m", c=KC)
                psum = ps.tile([O_SH, B_SH], F32, tag="ps", name="ps")
                nmm = 1 if KABLATE == "mm1" else KC
                if KABLATE != "nomm":
                    for c in range(nmm):
                        nc.tensor.matmul(
                            psum[:, :],
                            e3[:, c, B_SH : B_SH + O_SH],
                            e3[:, c, 0:B_SH],
                            start=(c == 0),
                            stop=(c == nmm - 1),
                        )
                return psum

            def back(psum):
                nc.gpsimd.tensor_scalar_add(counter[:, :], counter[:, :], 1.0)
                if KABLATE == "empty":
                    return
                osb = op.tile([O_SH, B_SH], F32, tag="osb", name="osb")
                if KLOG == "bits":
                    # Schraudolph log: ln(p) ~ (bits(p)*2^-23 - 127 + 0.043)*ln2
                    # (max err 0.03 in ln units -> 0.0012 after /T). Fused
                    # with the /T + C affine into ONE vector op on the raw
                    # psum bit pattern.
                    ln2 = float(np.log(2.0))
                    s1 = ln2 / (T * (1 << 23))
                    s2 = (0.0430357 - 127.0) * (1 << 23) * s1 + C
                    nc.vector.tensor_scalar(
                        osb[:, :], psum[:, :].bitcast(mybir.dt.int32),
                        s1, s2, MULT, ADD,
                    )
                else:
                    lnsb = op.tile([O_SH, B_SH], F32, tag="ln", name="ln")
                    nc.scalar.activation(lnsb[:, :], psum[:, :], LN)
                    if KAFF == "act":
                        nc.scalar.activation(
                            osb[:, :], lnsb[:, :], COPY, bias=C, scale=1.0 / T
                        )
                    else:
                        nc.vector.tensor_scalar(
                            osb[:, :], lnsb[:, :], 1.0 / T, C, MULT, ADD
                        )
                if KABLATE != "noout":
                    odma_eng = nc.gpsimd if KODMAENG == "gpsimd" else nc.sync
                    odma_eng.dma_start(out=out[:, :], in_=osb[:, :])

            if nrep == 1:
                back(front())
            else:
                assert nrep % BODY_UNROLL == 0, f"nrep must be divisible by {BODY_UNROLL}"
                with tc.For_i(0, nrep // BODY_UNROLL, staggered_reset=bool(KSTAG)):
                    if KPIPE:
                        # Software-pipeline the unrolled bodies: emit body
                        # k+1's front (DMA/exp/matmuls) before body k's back
                        # (ln/affine/out-DMA) so the in-order Scalar engine
                        # never stalls on the Tensor engine between its
                        # exp(k) and ln(k).
                        psum = front()
                        for _ in range(BODY_UNROLL - 1):
                            nxt = front()
                            back(psum)
                            psum = nxt
                        back(psum)
                    else:
                        for _ in range(BODY_UNROLL):
                            back(front())
            nc.sync.dma_start(out=iters[:, :], in_=counter[:, :])

    nc.compile()
    return nc


_NC = None


def _get_nc():
    global _NC
    if _NC is None:
        _NC = build_nc()
    return _NC


def core_slices(k: int):
    """(o0, b0) for core k: o-shard-major over a NO_SH x NB_SH grid."""
    ob, bb = k % NO_SH, k // NO_SH
    return ob * O_SH, bb * B_SH


def make_in_maps(x: np.ndarray, W: np.ndarray):
    x = np.asarray(x, dtype=np.float32)
    W = np.asarray(W, dtype=np.float32)
    # Clamp: terms that far below a row's max can't influence the result
    # (< e^-17 relative); the clamp keeps the Schraudolph-exp int16 bit
    # arithmetic in range and every downstream float normal (no denormals).
    xs = np.maximum(x.T - C, XCLAMP).astype(np.float16)  # [IN, B]
    Ws = W.T.astype(np.float16)  # [IN, OUT]
    maps = []
    for k in range(NCORES):
        o0, b0 = core_slices(k)
        xT = xs[:, b0 : b0 + B_SH].reshape(KC, 128, B_SH).transpose(1, 0, 2)
        wT = Ws[:, o0 : o0 + O_SH].reshape(KC, 128, O_SH).transpose(1, 0, 2)
        inh = np.concatenate([xT, wT], axis=2).reshape(128, KC * M)
        maps.append({"inh": np.ascontiguousarray(inh)})
    return maps


def kernel(x, W, trace: bool = False):
    nc = _get_nc()
    res = run_bass_kernel_spmd(
        nc, make_in_maps(x, W), core_ids=list(range(NCORES)), trace=trace
    )
    out = np.empty((B, OUT), np.float32)
    for k in range(NCORES):
        o0, b0 = core_slices(k)
        out[b0 : b0 + B_SH, o0 : o0 + O_SH] = res.results[k]["out"].T
    if trace:
        return out, res
    return out


# revision 48
# speedup vs baseline: 1.3861x; 1.3861x over previous
"""Tropical (max-plus) linear kernel for Trainium2 via log-sum-exp matmul.

out[b, o] = max_i (W[o, i] + x[b, i]),  x: [512, 1024] f32, W: [512, 1024] f32.

Identity: max_i(W+x) = (1/t)*log(sum_i e^{t(x-c)} * e^{tW}) + c - smoothing,
so the max-plus contraction becomes a real bf16 GEMM on the Tensor engine
instead of a broadcast-add + reduce-max on the Vector engine (the 792us
baseline). Smoothing error <= ln(#near-ties)/t; with t=25, c=4 the exact
end-to-end numerics (fp16 inputs, bf16 exp, f32 psum) give max abs err 0.063
vs the 0.108 tolerance (2e-2 * absmax), verified against the reference on
all 512x512 outputs. The c-shift keeps e^{t(x-c)} inside bf16 range for
every input that can influence a row max; it is folded into the host-side
fp16 packing (x - 4.0) and added back by the final affine.

Sharding (8 NeuronCores, SPMD): grid of NO_SH out-shards x NB_SH batch-shards
(NO_SH*NB_SH = 8; default 4x2). Host packs each core's x/W slices k-major-
transposed into one fp16 tensor and reassembles the per-core [O_SH, B_SH]
outputs.

Per-core body (~12 instructions, default config):
  DMA in  : packed [128, KC, B_SH+O_SH] f16, split over 2 DGE queues
  Vector  : ee = Schraudolph exp: ONE int16 tensor_scalar writes the bf16
            BIT PATTERNS of e^{T*in} directly (bits = z*128 + 16250.5,
            ~3% rel err -> +-0.0024 on the output after /T)
  Tensor  : psum[O_SH, B_SH] = sum_c ee[:,c,W-part].T @ ee[:,c,x-part]
  Vector  : Schraudolph log fused with /T + C: ONE tensor_scalar on the
            int32-bitcast psum;  DMA out [O_SH, B_SH] f32 on gpsimd queue

Engine-queue discipline matters most: the out-DMA must NOT share the SP DMA
queue with the in-DMA, or head-of-line blocking serializes iterations
(measured 15.4 us vs 4 us). The Scalar-engine Exp/Ln path (KEXP=act,
KLOG=ln) is kept as a fallback config; the bit-trick path frees the Scalar
engine entirely and fuses the whole epilogue into one Vector op.

Timing note: on this axon-proxied setup, per-call wall time scales with NEFF
size (payload upload), so python-unrolled nrep-differencing measures upload
cost (~1.2 ms/"iter"), not device time. build_nc(nrep>1) therefore wraps the
body in a tc.For_i hardware loop (constant NEFF size; trip count differencing
isolates true device exec time). The body is unrolled BODY_UNROLL x inside
the loop so the per-trip all-engine barrier (~10 us) amortizes away.
"""

import os

import numpy as np  # noqa: E402

import concourse.bacc as bacc
import concourse.tile as tile
from concourse import mybir
from concourse.bass_utils import run_bass_kernel_spmd

B, IN, OUT = 512, 1024, 512
NCORES = 8
KC = IN // 128  # 8 k-chunks of 128 partitions

T = 25.0
# c=3.0 with the -2.0 clamp keeps every exp value, bf16 product, and f32
# psum term in NORMAL float range (min product ~2e-27 vs denormal threshold
# 1.2e-38) -- denormal operands measurably slow the PE on this hardware.
C = float(os.environ.get("KC0", "3.0"))
XCLAMP = float(os.environ.get("KCLAMP", "-2.0"))

F32 = mybir.dt.float32
F16 = mybir.dt.float16
BF16 = mybir.dt.bfloat16
EXP = mybir.ActivationFunctionType.Exp
LN = mybir.ActivationFunctionType.Ln
COPY = mybir.ActivationFunctionType.Copy
MULT = mybir.AluOpType.mult
ADD = mybir.AluOpType.add

BODY_UNROLL = int(os.environ.get("KUNROLL", "16"))
KBUFS = int(os.environ.get("KBUFS", "4"))
KPSBUFS = int(os.environ.get("KPSBUFS", "8"))
KAFF = os.environ.get("KAFF", "act")  # dve | act (only used with KLOG=ln)
KSPLIT = int(os.environ.get("KSPLIT", "2"))  # in-DMA split count
# Out-DMA queue: "scalar" = Activation HWDGE (fast; SP would head-of-line
# block the in-DMA, gpsimd goes through the ~1us-overhead software DGE).
KODMAENG = os.environ.get("KODMAENG", "scalar")  # sync | gpsimd | scalar
KPIPE = int(os.environ.get("KPIPE", "0"))  # software-pipeline unrolled bodies
KLOG = os.environ.get("KLOG", "bits")  # ln | bits (Schraudolph log on DVE)
KEXP = os.environ.get("KEXP", "split")  # act | dve | dvew | split
KSHARD = os.environ.get("KSHARD", "o4b2")  # o8 | o4b2
KABLATE = os.environ.get("KABLATE", "")  # nodma|noexp|nomm|noout|empty (perf probes)
KSTAG = int(os.environ.get("KSTAG", "0"))  # For_i staggered_reset
KDQ = os.environ.get("KDQ", "ss")  # in-DMA queue rotation: ss=sync/scalar, sg=sync/gpsimd
KLOGENG = os.environ.get("KLOGENG", "dve")  # dve | gpsimd (engine for KLOG=bits)
KSPLITX = int(os.environ.get("KSPLITX", "4"))  # k-chunks on ACT in KEXP=split

NO_SH, NB_SH = (8, 1) if KSHARD == "o8" else (4, 2)
O_SH = OUT // NO_SH
B_SH = B // NB_SH
M = B_SH + O_SH  # packed columns per k-chunk

# legacy name used by test.py's sim path
O_PER_CORE = O_SH


def build_nc(nrep: int = 1) -> bacc.Bacc:
    nc = bacc.Bacc("TRN2", num_devices=NCORES)
    # inh[p, c*M + b]      = f16(x[b0 + b, c*128 + p] - C)   b in [0, B_SH)
    # inh[p, c*M + B_SH+o] = f16(W[o0 + o, c*128 + p])       o in [0, O_SH)
    inh = nc.dram_tensor("inh", [128, KC * M], F16, kind="ExternalInput")
    out = nc.dram_tensor("out", [O_SH, B_SH], F32, kind="ExternalOutput")
    # Proof the timing loop really ran: per-iteration counter, read back by
    # the harness and checked against nrep (the body itself is idempotent,
    # so output correctness alone can't detect a broken/short loop).
    iters = nc.dram_tensor("iters", [1, 1], F32, kind="ExternalOutput")

    with tile.TileContext(nc) as tc:
        with (
            tc.tile_pool(name="cnt", bufs=1) as cnt,
            tc.tile_pool(name="ip", bufs=KBUFS) as ip,
            tc.tile_pool(name="ep", bufs=KBUFS) as ep,
            tc.tile_pool(name="op", bufs=KBUFS) as op,
            tc.tile_pool(name="ps", bufs=KPSBUFS, space="PSUM") as ps,
        ):
            counter = cnt.tile([1, 1], F32, tag="cnt", name="cnt")
            nc.gpsimd.memset(counter[:, :], 0.0)

            def front():
                if KABLATE == "empty":
                    return None
                ts = ip.tile([128, KC * M], F16, tag="ts", name="ts")
                if KABLATE == "tinydma":
                    nc.sync.dma_start(out=ts[:, 0:128], in_=inh[:, 0:128])
                elif KABLATE == "nodma":
                    pass
                elif KSPLIT == 1:
                    nc.sync.dma_start(out=ts, in_=inh[:, :])
                else:
                    step = KC * M // KSPLIT
                    rot = (
                        [nc.sync, nc.gpsimd, nc.scalar]
                        if KDQ == "sg"
                        else [nc.sync, nc.scalar, nc.gpsimd]
                    )
                    for s in range(KSPLIT):
                        eng = rot[s % 3]
                        eng.dma_start(
                            out=ts[:, s * step : (s + 1) * step],
                            in_=inh[:, s * step : (s + 1) * step],
                        )
                ee = ep.tile([128, KC * M], BF16, tag="ee", name="ee")
                if KABLATE == "noexp":
                    ee = ts.bitcast(BF16) if hasattr(ts, "bitcast") else ts
                    ee3 = ts[:, :].rearrange("p (c m) -> p c m", c=KC)
                    psum = ps.tile([O_SH, B_SH], F32, tag="ps", name="ps")
                    for c in range(KC):
                        nc.tensor.matmul(
                            psum[:, :], ee3[:, c, B_SH : B_SH + O_SH],
                            ee3[:, c, 0:B_SH], start=(c == 0), stop=(c == KC - 1),
                        )
                    return psum
                if KEXP == "act":
                    nc.scalar.activation(ee[:, :], ts[:, :], EXP, bias=0.0, scale=T)
                elif KEXP in ("split", "split3"):
                    # Split the exp wall time across engines: Scalar does the
                    # leading k-chunks with exact table exp while Vector (and
                    # for split3, GpSimd) do the rest with the Schraudolph
                    # bit trick, all in parallel.
                    h = KSPLITX * M if KEXP == "split" else 3 * M
                    nc.scalar.activation(
                        ee[:, 0:h], ts[:, 0:h], EXP, bias=0.0, scale=T
                    )
                    es1 = float(T * np.log2(np.e) * 128.0)
                    es2 = float((127.0 - 0.0430357) * 128.0)
                    eei = ee[:, :].bitcast(mybir.dt.int16)
                    h2 = KC * M if KEXP == "split" else 6 * M
                    nc.vector.tensor_scalar(
                        eei[:, h:h2], ts[:, h:h2], es1, es2, MULT, ADD
                    )
                    if KEXP == "split3":
                        nc.gpsimd.tensor_scalar(
                            eei[:, h2 : KC * M], ts[:, h2 : KC * M],
                            es1, es2, MULT, ADD,
                        )
                else:
                    # Schraudolph exp, built directly as bf16 bit patterns:
                    # e^{T*y} = 2^{T*y*log2(e)}; bf16 bits ~ z*128 + (127-m)*128
                    # (~3% rel err -> +-0.0024 after the final /T; host packing
                    # clamps x-C at -3.4 so the int16 range can't wrap).
                    es1 = float(T * np.log2(np.e) * 128.0)
                    es2 = float((127.0 - 0.0430357) * 128.0)
                    eei = ee[:, :].bitcast(mybir.dt.int16)
                    if KEXP == "dve":
                        nc.vector.tensor_scalar(
                            eei, ts[:, :], es1, es2, MULT, ADD
                        )
                    else:  # dvew: x-part exact on ACT, W-part Schraudolph on DVE
                        t3 = ts[:, :].rearrange("p (c m) -> p c m", c=KC)
                        x3 = ee[:, :].rearrange("p (c m) -> p c m", c=KC)
                        nc.scalar.activation(
                            x3[:, :, 0:B_SH], t3[:, :, 0:B_SH], EXP,
                            bias=0.0, scale=T,
                        )
                        w3 = eei.rearrange("p (c m) -> p c m", c=KC)
                        nc.vector.tensor_scalar(
                            w3[:, :, B_SH : B_SH + O_SH],
                            t3[:, :, B_SH : B_SH + O_SH],
                            es1, es2, MULT, ADD,
                        )
                e3 = ee[:, :].rearrange("p (c m) -> p c m", c=KC)
                psum = ps.tile([O_SH, B_SH], F32, tag="ps", name="ps")
                nmm = 1 if KABLATE == "mm1" else KC
                if KABLATE != "nomm":
                    for c in range(nmm):
                        nc.tensor.matmul(
                            psum[:, :],
                            e3[:, c, B_SH : B_SH + O_SH],
                            e3[:, c, 0:B_SH],
                            start=(c == 0),
                            stop=(c == nmm - 1),
                        )
                return psum

            def back(psum):
                nc.gpsimd.tensor_scalar_add(counter[:, :], counter[:, :], 1.0)
                if KABLATE == "empty":
                    return
                osb = op.tile([O_SH, B_SH], F32, tag="osb", name="osb")
                if KLOG == "bits":
                    # Schraudolph log: ln(p) ~ (bits(p)*2^-23 - 127 + 0.043)*ln2
                    # (max err 0.03 in ln units -> 0.0012 after /T). Fused
                    # with the /T + C affine into ONE vector op on the raw
                    # psum bit pattern.
                    ln2 = float(np.log(2.0))
                    s1 = ln2 / (T * (1 << 23))
                    s2 = (0.0430357 - 127.0) * (1 << 23) * s1 + C
                    log_eng = nc.gpsimd if KLOGENG == "gpsimd" else nc.vector
                    log_eng.tensor_scalar(
                        osb[:, :], psum[:, :].bitcast(mybir.dt.int32),
                        s1, s2, MULT, ADD,
                    )
                else:
                    lnsb = op.tile([O_SH, B_SH], F32, tag="ln", name="ln")
                    nc.scalar.activation(lnsb[:, :], psum[:, :], LN)
                    if KAFF == "act":
                        nc.scalar.activation(
                            osb[:, :], lnsb[:, :], COPY, bias=C, scale=1.0 / T
                        )
                    else:
                        nc.vector.tensor_scalar(
                            osb[:, :], lnsb[:, :], 1.0 / T, C, MULT, ADD
                        )
                if KABLATE != "noout":
                    odma_eng = {
                        "gpsimd": nc.gpsimd,
                        "scalar": nc.scalar,
                    }.get(KODMAENG, nc.sync)
                    odma_eng.dma_start(out=out[:, :], in_=osb[:, :])

            if nrep == 1:
                back(front())
            else:
                assert nrep % BODY_UNROLL == 0, f"nrep must be divisible by {BODY_UNROLL}"
                with tc.For_i(0, nrep // BODY_UNROLL, staggered_reset=bool(KSTAG)):
                    if KPIPE:
                        # Software-pipeline the unrolled bodies: emit body
                        # k+1's front (DMA/exp/matmuls) before body k's back
                        # (ln/affine/out-DMA) so the in-order Scalar engine
                        # never stalls on the Tensor engine between its
                        # exp(k) and ln(k).
                        psum = front()
                        for _ in range(BODY_UNROLL - 1):
                            nxt = front()
                            back(psum)
                            psum = nxt
                        back(psum)
                    else:
                        for _ in range(BODY_UNROLL):
                            back(front())
            nc.sync.dma_start(out=iters[:, :], in_=counter[:, :])

    nc.compile()
    return nc


_NC = None


def _get_nc():
    global _NC
    if _NC is None:
        _NC = build_nc()
    return _NC


def core_slices(k: int):
    """(o0, b0) for core k: o-shard-major over a NO_SH x NB_SH grid."""
    ob, bb = k % NO_SH, k // NO_SH
    return ob * O_SH, bb * B_SH


def make_in_maps(x: np.ndarray, W: np.ndarray):
    x = np.asarray(x, dtype=np.float32)
    W = np.asarray(W, dtype=np.float32)
    # Clamp: terms that far below a row's max can't influence the result
    # (< e^-17 relative); the clamp keeps the Schraudolph-exp int16 bit
    # arithmetic in range and every downstream float normal (no denormals).
    xs = np.maximum(x.T - C, XCLAMP).astype(np.float16)  # [IN, B]
    Ws = W.T.astype(np.float16)  # [IN, OUT]
    maps = []
    for k in range(NCORES):
        o0, b0 = core_slices(k)
        xT = xs[:, b0 : b0 + B_SH].reshape(KC, 128, B_SH).transpose(1, 0, 2)
        wT = Ws[:, o0 : o0 + O_SH].reshape(KC, 128, O_SH).transpose(1, 0, 2)
        inh = np.concatenate([xT, wT], axis=2).reshape(128, KC * M)
        maps.append({"inh": np.ascontiguousarray(inh)})
    return maps


def kernel(x, W, trace: bool = False):
    nc = _get_nc()
    res = run_bass_kernel_spmd(
        nc, make_in_maps(x, W), core_ids=list(range(NCORES)), trace=trace
    )
    out = np.empty((B, OUT), np.float32)
    for k in range(NCORES):
        o0, b0 = core_slices(k)
        out[b0 : b0 + B_SH, o0 : o0 + O_SH] = res.results[k]["out"].T
    if trace:
        return out, res
    return out
